# revision 18
# baseline (speedup 1.0000x reference)
"""GCN critic network kernel for 8 TRN2 NeuronCores.

Strategy (dst-shard graph parallel, bf16 gather table, pipelined AllGather):
  - Nodes are permuted host-side into (core, block, slot) bins balancing
    per-block in-degree for each of 4 table quarters (minimizes padding).
  - Each core computes y = dinv * (x @ Wg.T) bf16 for its slice; the slice
    is split into 4 quarter-tables, each AllGathered separately so pass-q
    gather/compute overlaps the AllGather of quarter q+1.
  - Per dst block, segment-sum via one-hot matmuls: one-hot built with
    tensor_scalar is_equal (iota row vs per-edge dst-slot scalar, 4x DVE
    mode); gathered y rows are the moving operand.
  - Self loops are an identity-matmul accumulate of local y (never
    gathered); cross-pass partials park in SBUF (Sacc) via identity-matmul
    reload.
  - v = colsum(relu(.)) via ones-matmul + colsum(x) via DVE row-reduce;
    AllReduce [1,128]; tiny MLP with host-pretransposed weights.
"""

import os
import math
import numpy as np
import ml_dtypes

BF16 = ml_dtypes.bfloat16
N = 50000
E = 800000
D = 128
NCORES = 8
NB = 49             # blocks per core
NPC = 6250          # real nodes per core
NPAD = NB * 128     # padded positions per core (6272)
NPASS = 4
QSTART = [0, 12, 24, 36]          # first block of each quarter
QNB = [12, 12, 12, 13]            # blocks per quarter
QROWS = [q * 128 for q in QNB]    # rows per core per quarter
SEGC = int(os.environ.get("KB_SEGC", "8"))    # chunks per gather segment
                                              # (dma_gather hangs above 1024 idxs)
DDS = int(os.environ.get("KB_DDS", "98304"))
NQ = int(os.environ.get("KB_QUEUES", "1"))
PADSLOT = 300.0     # dst-slot sentinel for padding edges (no one-hot match)

DEBUG_BLOCKS = (int(os.environ["KB_DEBUG_BLOCKS"])
                if "KB_DEBUG_BLOCKS" in os.environ else None)
SKIP_MLP = bool(os.environ.get("KB_SKIP_MLP"))
SKIP_MAIN = bool(os.environ.get("KB_SKIP_MAIN"))


def _pack_nodes(inq, classq):
    """Pack nodes into (core, block, slot) bins against per-(pass, block)
    edge-count caps chosen near the ideal chunk budget (cross-core max fill
    sets the uniform chunk count, so bins aim for ct*128 ceilings rather
    than a uniform mean). classq[n] = quarter whose blocks node n must land
    in. Returns c_of, b_of, s_of."""
    c_of = np.empty(N, np.int64)
    b_of = np.empty(N, np.int64)
    s_of = np.empty(N, np.int64)

    for q in range(NPASS):
        nbq = QNB[q]
        nbins = NCORES * nbq
        cap_n = np.full(nbins, 128, np.int64)
        if q == NPASS - 1:
            cap_n[nbq - 1::nbq] = 106          # block 48 of each core
        nodes = np.nonzero(classq == q)[0]
        w = inq[:, nodes].sum(axis=0)
        nodes = nodes[np.argsort(-w, kind="stable")]
        fills = np.zeros((NPASS, nbins))
        cnt = np.zeros(nbins, np.int64)
        tgt = np.maximum(inq.sum(axis=1) / (NCORES * NB), 1.0)  # [NPASS]
        for n in nodes:
            score = ((fills + inq[:, n][:, None]) / tgt[:, None]).max(axis=0)
            score[cnt >= cap_n] = np.inf
            j = int(np.argmin(score))
            c_of[n] = j // nbq
            b_of[n] = QSTART[q] + j % nbq
            s_of[n] = cnt[j]
            fills[:, j] += inq[:, n]
            cnt[j] += 1
    return c_of, b_of, s_of


def _prep(edge_index):
    """Host-side sharding prep. Returns per-core inputs + uniform plan."""
    src = np.asarray(edge_index[0]).astype(np.int64)
    dst = np.asarray(edge_index[1]).astype(np.int64)

    deg_in = np.bincount(dst, minlength=N)
    dinv = (1.0 / np.sqrt(deg_in + 1.0)).astype(np.float32)

    # initial quarter classes: random, sized to real slots per quarter
    sizes = [NCORES * QNB[q] * 128 for q in range(NPASS)]
    sizes[-1] -= NCORES * 22                   # block 48 holds 106 real
    rng = np.random.RandomState(12345)
    perm0 = rng.permutation(N)
    classq = np.empty(N, np.int64)
    p0 = 0
    for q in range(NPASS):
        classq[perm0[p0:p0 + sizes[q]]] = q
        p0 += sizes[q]

    # inq[q, n] = # in-edges of n from quarter-q sources
    inq = np.zeros((NPASS, N), np.int64)
    for q in range(NPASS):
        np.add.at(inq[q], dst[classq[src] == q], 1)

    c_of, b_of, s_of = _pack_nodes(inq, classq)

    # per-quarter table rows (concat over cores)
    qq = classq                                 # quarter of node (as src)
    bq = b_of - np.asarray(QSTART)[qq]          # block within quarter
    row = c_of * np.asarray(QROWS)[qq] + bq * 128 + s_of

    cnt = np.zeros((NCORES, NPASS, NB), np.int64)
    np.add.at(cnt, (c_of[dst], qq[src], b_of[dst]), 1)
    ct = np.ceil(cnt.max(axis=0) / 128.0).astype(np.int64)   # [NPASS, NB]
    ct = np.maximum(ct, 1)
    CH = ct.sum(axis=1)                                      # [NPASS]
    off = np.zeros((NPASS, NB), np.int64)
    off[:, 1:] = np.cumsum(ct, axis=1)[:, :-1]

    def wrap_idx(loc):
        nch = len(loc) // 128
        cols = []
        for s0 in range(0, nch, SEGC):
            seg = loc[s0 * 128: min(nch, s0 + SEGC) * 128]
            a = seg.reshape(-1, 16).T.astype(np.int16)
            cols.append(np.tile(a, (8, 1)))
        return np.concatenate(cols, axis=1)

    in_extra = []
    for c in range(NCORES):
        m = c_of[dst] == c
        es, ed = src[m], dst[m]
        eq = qq[es]
        d = {}
        for P in range(NPASS):
            sel = eq == P
            erow = row[es[sel]]
            eb = b_of[ed[sel]]
            eslot = s_of[ed[sel]]
            order = np.argsort(eb, kind="stable")
            erow, eb, eslot = erow[order], eb[order], eslot[order]
            nslots = int(CH[P]) * 128
            loc = np.zeros(nslots, np.int64)
            dsb = np.full(nslots, PADSLOT, np.float64)
            bstart = np.zeros(NB, np.int64)
            bstart[1:] = np.cumsum(np.bincount(eb, minlength=NB))[:-1]
            pos = off[P][eb] * 128 + (np.arange(len(eb)) - bstart[eb])
            loc[pos] = erow
            dsb[pos] = eslot
            d[f"idx{P}"] = wrap_idx(loc)
            # position i -> (partition i%128, col i//128)
            d[f"dsb{P}"] = np.ascontiguousarray(dsb.reshape(-1, 128).T
                                               ).astype(np.float32)
        in_extra.append(d)

    xt = np.zeros((NCORES, 128, NPAD), dtype=BF16)
    dv = np.zeros((NCORES, 128, NB), np.float32)

    def fill_x(x):
        pos = c_of * NPAD + b_of * 128 + s_of
        xt_flat = np.zeros((NCORES * NPAD, D), np.float32)
        xt_flat[pos] = x
        xtv = xt_flat.reshape(NCORES, NPAD, D)
        for c in range(NCORES):
            xt[c] = np.ascontiguousarray(xtv[c].T).astype(BF16)
        dvf = np.zeros(NCORES * NPAD, np.float32)
        dvf[pos] = dinv
        for c in range(NCORES):
            dv[c] = dvf[c * NPAD:(c + 1) * NPAD].reshape(NB, 128).T

    plan = {"ct": ct, "CH": CH, "off": off}
    return dinv, in_extra, plan, fill_x, xt, dv


def _build(plan, bias_info, probe=False):
    import concourse.bacc as bacc
    import concourse.tile as tile
    from concourse import mybir

    f32 = mybir.dt.float32
    bf16 = mybir.dt.bfloat16
    i16 = mybir.dt.int16
    Alu = mybir.AluOpType
    Act = mybir.ActivationFunctionType
    Ax = mybir.AxisListType

    ct, CH, off = plan["ct"], plan["CH"], plan["off"]
    has_bg, has_b1, has_b2, b3val = bias_info

    nc = bacc.Bacc("TRN2", target_bir_lowering=False, debug=False,
                   num_devices=(1 if probe else NCORES),
                   num_swdge_queues=NQ,
                   dynamic_dma_scratch_size=DDS)

    def din(name, shape, dt=f32):
        return nc.dram_tensor(name, list(shape), dt, kind="ExternalInput")

    xt_d = din("xt", [128, NPAD], bf16)
    dinv_d = din("dinvc", [128, NB])
    iota_d = din("iota2d", [128, 128], bf16)
    ones_d = din("onesh", [128, 1], bf16)
    idf_d = din("idf", [128, 128])
    idh_d = din("idh", [128, 128], bf16)
    wgT_d = din("wgT", [128, 128], bf16)
    w1T_d = din("w1T", [128, 512])
    w2T_d = din("w2T", [128, 4, 256])
    w3T_d = din("w3T", [128, 2])
    idx_d = [din(f"idx{P}", [128, int(CH[P]) * 8], i16) for P in range(NPASS)]
    dsb_d = [din(f"dsb{P}", [128, int(CH[P])]) for P in range(NPASS)]
    bg_d = din("bgt", [128, 128]) if has_bg else None
    b1_d = din("b1c", [128, 4]) if has_b1 else None
    b2_d = din("b2c", [128, 2]) if has_b2 else None
    out_d = nc.dram_tensor("out", [1, 1], f32, kind="ExternalOutput")

    ys = [nc.dram_tensor(f"ys{q}", [QROWS[q], D], bf16) for q in range(NPASS)]
    yf = [nc.dram_tensor(f"yf{q}", [QROWS[q] * NCORES, D], bf16,
                         addr_space="Shared") for q in range(NPASS)]
    vb = nc.dram_tensor("vb", [1, 128], f32)
    vr = nc.dram_tensor("vr", [1, 128], f32, addr_space="Shared")

    RG = [list(range(NCORES))]

    with tile.TileContext(nc) as tc:
        with (
            tc.tile_pool(name="const", bufs=1) as cpool,
            tc.tile_pool(name="xt", bufs=1) as xtpool,
            tc.tile_pool(name="y8", bufs=1) as ypool,
            tc.tile_pool(name="sacc", bufs=1) as apool,
            tc.tile_pool(name="seg", bufs=(2 if SEGC > 8 else 4)) as segpool,
            tc.tile_pool(name="oh", bufs=(2 if SEGC > 8 else 4)) as ohpool,
            tc.tile_pool(name="hb", bufs=3) as hpool,
            tc.tile_pool(name="mlp", bufs=1) as mpool,
            tc.tile_pool(name="psy", bufs=2, space="PSUM") as pbpool,
            tc.tile_pool(name="psS", bufs=3, space="PSUM") as pspool,
            tc.tile_pool(name="psv", bufs=1, space="PSUM") as pvpool,
            tc.tile_pool(name="psT", bufs=1, space="PSUM") as ptpool,
        ):
            # ---- constants ----
            wgT_t = cpool.tile([128, 128], bf16, tag="wgT")
            nc.sync.dma_start(wgT_t[:], wgT_d[:])
            dinv_t = cpool.tile([128, NB], f32, tag="dinv")
            nc.sync.dma_start(dinv_t[:], dinv_d[:])
            iota_t = cpool.tile([128, 128], bf16, tag="iota")
            nc.sync.dma_start(iota_t[:], iota_d[:])
            ones_t = cpool.tile([128, 1], bf16, tag="ones")
            nc.sync.dma_start(ones_t[:], ones_d[:])
            idf_t = cpool.tile([128, 128], f32, tag="idf")
            nc.sync.dma_start(idf_t[:], idf_d[:])
            idh_t = cpool.tile([128, 128], bf16, tag="idh")
            nc.sync.dma_start(idh_t[:], idh_d[:])
            if has_bg:
                bg_t = cpool.tile([128, 128], f32, tag="bgt")
                nc.sync.dma_start(bg_t[:], bg_d[:])

            # ---- phase B: y = dinv * (x @ Wg.T) bf16, 4 quarters ----
            xT = xtpool.tile([128, NPAD], bf16)
            nc.sync.dma_start(xT[:, 0:QSTART[2] * 128],
                              xt_d[:, 0:QSTART[2] * 128])
            nc.sync.dma_start(xT[:, QSTART[2] * 128:NPAD],
                              xt_d[:, QSTART[2] * 128:NPAD])
            y8 = ypool.tile([128, NPAD], bf16)

            qend = [QSTART[q] + QNB[q] for q in range(NPASS)]
            for b in range(NB):
                psy = pbpool.tile([128, 128], f32, tag="psy")
                nc.tensor.matmul(psy[:], xT[:, b * 128:(b + 1) * 128],
                                 wgT_t[:], start=True, stop=True)
                nc.scalar.activation(y8[:, b * 128:(b + 1) * 128], psy[:],
                                     Act.Copy, scale=dinv_t[:, b:b + 1])
                q = [qi for qi in range(NPASS) if qend[qi] == b + 1]
                if q:
                    q = q[0]
                    r0 = QSTART[q] * 128
                    nc.sync.dma_start(
                        ys[q][:].rearrange("(b p) d -> p b d", p=128),
                        y8[:, r0:r0 + QROWS[q]]
                        .rearrange("p (b d) -> p b d", d=128))
                    if probe:
                        nc.gpsimd.dma_start(
                            yf[q][0:QROWS[q], :], ys[q][:])
                    else:
                        nc.gpsimd.collective_compute(
                            "AllGather", Alu.bypass, replica_groups=RG,
                            ins=[ys[q][:]], outs=[yf[q][:]])

            # colsum(x) as a column (DVE row-reduce over the transpose)
            vx = mpool.tile([128, 1], f32, tag="vx")
            nc.vector.tensor_reduce(vx[:], xT[:], Ax.X, Alu.add)

            # ---- main gather + one-hot segment-sum, NPASS passes ----
            idx_t = []
            dsb_t = []
            for P in range(NPASS):
                it = cpool.tile([128, int(CH[P]) * 8], i16, tag=f"idx{P}")
                nc.sync.dma_start(it[:], idx_d[P][:])
                idx_t.append(it)
                dt_ = cpool.tile([128, int(CH[P])], f32, tag=f"dsb{P}")
                nc.sync.dma_start(dt_[:], dsb_d[P][:])
                dsb_t.append(dt_)

            Sacc = apool.tile([128, NPAD], f32)
            psv = pvpool.tile([1, 128], f32)

            nblk = (0 if SKIP_MAIN else
                    (NB if DEBUG_BLOCKS is None else DEBUG_BLOCKS))

            def run_pass(P):
                chp = int(CH[P])
                nseg = math.ceil(chp / SEGC)
                seg_tiles = [None] * nseg
                oh_tiles = [None] * nseg
                ptr = [0]

                def ensure(s):
                    while ptr[0] <= s:
                        si = ptr[0]
                        ncols = min(SEGC, chp - si * SEGC)
                        tl = segpool.tile([128, ncols, 128], bf16, tag="seg")
                        nc.gpsimd.dma_gather(
                            tl[:], yf[P][:],
                            idx_t[P][:, si * SEGC * 8:
                                     si * SEGC * 8 + ncols * 8],
                            num_idxs=ncols * 128, num_idxs_reg=ncols * 128,
                            elem_size=128, elem_step=128,
                            queue_num=(si % NQ))
                        oh = ohpool.tile([128, ncols, 128], bf16, tag="oh")
                        for cc in range(ncols):
                            nc.vector.tensor_scalar(
                                oh[:, cc, :], iota_t[:],
                                dsb_t[P][:, si * SEGC + cc:
                                         si * SEGC + cc + 1],
                                None, Alu.is_equal)
                        seg_tiles[si] = tl
                        oh_tiles[si] = oh
                        ptr[0] += 1

                for b in range(nblk):
                    psS = pspool.tile([128, 128], f32, tag="psS")
                    nmm = int(ct[P][b]) + 1
                    if P == 0:
                        # self loop: psS starts with local y rows
                        nc.tensor.matmul(psS[:], idh_t[:],
                                         y8[:, b * 128:(b + 1) * 128],
                                         start=True, stop=(nmm == 1))
                    else:
                        # reload the cross-pass partial
                        nc.tensor.matmul(psS[:], idf_t[:],
                                         Sacc[:, b * 128:(b + 1) * 128],
                                         start=True, stop=(nmm == 1))
                    k = 1
                    for j in range(int(ct[P][b])):
                        ci = int(off[P][b]) + j
                        s, col = divmod(ci, SEGC)
                        ensure(s)
                        nc.tensor.matmul(
                            psS[:], oh_tiles[s][:, col, :],
                            seg_tiles[s][:, col, :],
                            start=False, stop=(k == nmm - 1))
                        k += 1
                    if P < NPASS - 1:
                        nc.scalar.copy(Sacc[:, b * 128:(b + 1) * 128], psS[:])
                    else:
                        if has_bg:
                            tmp = hpool.tile([128, 128], f32, tag="tmp")
                            nc.vector.tensor_tensor(tmp[:], psS[:], bg_t[:],
                                                    Alu.add)
                            src_ap = tmp[:]
                        else:
                            src_ap = psS[:]
                        hb = hpool.tile([128, 128], bf16, tag="hbt")
                        nc.scalar.activation(hb[:], src_ap, Act.Relu,
                                             scale=dinv_t[:, b:b + 1])
                        nc.tensor.matmul(psv[:], ones_t[:], hb[:],
                                         start=(b == 0), stop=(b == nblk - 1),
                                         skip_group_check=True)

            for P in range(NPASS):
                run_pass(P)
            if nblk == 0:
                nc.tensor.matmul(psv[:], ones_t[:],
                                 ones_t[:].to_broadcast([128, 128]),
                                 start=True, stop=True,
                                 skip_group_check=True)

            # ---- v = colsum(h) + colsum(x); AllReduce ----
            vh = mpool.tile([1, 128], f32, tag="vh")
            nc.scalar.copy(vh[:], psv[:])
            pvx = ptpool.tile([1, 128], f32, tag="pst")
            nc.tensor.transpose(pvx[:], vx[:], idf_t[:])
            vrow = mpool.tile([1, 128], f32, tag="vrow")
            nc.vector.tensor_tensor(vrow[:], vh[:], pvx[:], Alu.add)
            nc.sync.dma_start(vb[:], vrow[:])
            if probe:
                nc.gpsimd.dma_start(vr[:], vb[:])
            else:
                nc.gpsimd.collective_compute(
                    "AllReduce", Alu.add, replica_groups=RG,
                    ins=[vb[:]], outs=[vr[:]])
            vfull = mpool.tile([1, 128], f32, tag="vfull")
            nc.sync.dma_start(vfull[:], vr[:])

            # ---- MLP head (host-pretransposed weights) ----
            if SKIP_MLP:
                nc.sync.dma_start(out_d[:], vfull[0:1, 0:1])
            else:
                w1T_t = cpool.tile([128, 512], f32, tag="w1T")
                nc.sync.dma_start(w1T_t[:], w1T_d[:])
                w2T_t = cpool.tile([128, 4, 256], f32, tag="w2T")
                nc.sync.dma_start(w2T_t[:], w2T_d[:])
                w3T_t = cpool.tile([128, 2], f32, tag="w3T")
                nc.sync.dma_start(w3T_t[:], w3T_d[:])
                if has_b1:
                    b1_t = cpool.tile([128, 4], f32, tag="b1c")
                    nc.sync.dma_start(b1_t[:], b1_d[:])
                if has_b2:
                    b2_t = cpool.tile([128, 2], f32, tag="b2c")
                    nc.sync.dma_start(b2_t[:], b2_d[:])

                pvc = ptpool.tile([128, 1], f32, tag="pst")
                nc.tensor.transpose(pvc[:], vfull[:], idf_t[0:1, 0:1])
                vcol = mpool.tile([128, 1], f32, tag="vcol")
                nc.vector.tensor_copy(vcol[:], pvc[:])

                a1 = []
                for m in range(4):
                    ps1 = ptpool.tile([128, 1], f32, tag="pst")
                    nc.tensor.matmul(ps1[:], w1T_t[:, m * 128:(m + 1) * 128],
                                     vcol[:], start=True, stop=True)
                    a1t = mpool.tile([128, 1], f32, tag=f"a1{m}")
                    if has_b1:
                        nc.scalar.activation(a1t[:], ps1[:], Act.Relu,
                                             bias=b1_t[:, m:m + 1])
                    else:
                        nc.scalar.activation(a1t[:], ps1[:], Act.Relu)
                    a1.append(a1t)

                a2 = []
                for m in range(2):
                    ps2 = ptpool.tile([128, 1], f32, tag="pst")
                    for kk in range(4):
                        nc.tensor.matmul(
                            ps2[:], w2T_t[:, kk, m * 128:(m + 1) * 128],
                            a1[kk][:], start=(kk == 0), stop=(kk == 3))
                    a2t = mpool.tile([128, 1], f32, tag=f"a2{m}")
                    if has_b2:
                        nc.scalar.activation(a2t[:], ps2[:], Act.Relu,
                                             bias=b2_t[:, m:m + 1])
                    else:
                        nc.scalar.activation(a2t[:], ps2[:], Act.Relu)
                    a2.append(a2t)

                ps3 = ptpool.tile([1, 1], f32, tag="pst3")
                for kk in range(2):
                    nc.tensor.matmul(ps3[:], w3T_t[:, kk:kk + 1], a2[kk][:],
                                     start=(kk == 0), stop=(kk == 1))
                ot = mpool.tile([1, 1], f32, tag="ot")
                nc.scalar.activation(ot[:], ps3[:], Act.Copy,
                                     bias=float(b3val))
                nc.sync.dma_start(out_d[:], ot[:])

    nc.compile()
    return nc


TRACE = False
LAST_EXEC_NS = None
LAST_RESULT = None


def kernel(**inputs):
    from concourse.bass_utils import run_bass_kernel_spmd

    x = np.asarray(inputs["x"], dtype=np.float32)
    Wg = np.asarray(inputs["Wg"], dtype=np.float32)
    bg = np.asarray(inputs["bg"], dtype=np.float32)
    W1 = np.asarray(inputs["W1"], dtype=np.float32)
    b1 = np.asarray(inputs["b1"], dtype=np.float32)
    W2 = np.asarray(inputs["W2"], dtype=np.float32)
    b2 = np.asarray(inputs["b2"], dtype=np.float32)
    W3 = np.asarray(inputs["W3"], dtype=np.float32)
    b3 = np.asarray(inputs["b3"], dtype=np.float32)

    dinv, in_extra, plan, fill_x, xt, dv = _prep(inputs["edge_index"])
    fill_x(x)
    bias_info = (bool(bg.any()), bool(b1.any()), bool(b2.any()),
                 float(b3.reshape(-1)[0]))
    nc = _build(plan, bias_info)

    iota = np.tile(np.arange(128, dtype=np.float32)[None, :],
                   (128, 1)).astype(BF16)
    idf = np.eye(128, dtype=np.float32)
    idh = np.eye(128, dtype=np.float32).astype(BF16)
    ones = np.ones((128, 1), dtype=np.float32).astype(BF16)
    w1T = np.ascontiguousarray(W1.T)
    w2T = np.ascontiguousarray(W2.T).reshape(4, 128, 256).transpose(1, 0, 2)
    w2T = np.ascontiguousarray(w2T)
    w3T = np.ascontiguousarray(W3.reshape(256)).reshape(2, 128).T
    w3T = np.ascontiguousarray(w3T)

    in_maps = []
    for c in range(NCORES):
        m = {"xt": xt[c], "dinvc": dv[c], "iota2d": iota, "onesh": ones,
             "idf": idf, "idh": idh,
             "wgT": np.ascontiguousarray(Wg.T).astype(BF16),
             "w1T": w1T, "w2T": w2T, "w3T": w3T}
        for P in range(NPASS):
            m[f"idx{P}"] = in_extra[c][f"idx{P}"]
            m[f"dsb{P}"] = in_extra[c][f"dsb{P}"]
        if bias_info[0]:
            m["bgt"] = np.tile(bg.reshape(1, 128), (128, 1))
        if bias_info[1]:
            m["b1c"] = np.ascontiguousarray(b1.reshape(4, 128).T)
        if bias_info[2]:
            m["b2c"] = np.ascontiguousarray(b2.reshape(2, 128).T)
        in_maps.append(m)

    res = run_bass_kernel_spmd(nc, in_maps, list(range(NCORES)), trace=TRACE)
    global LAST_EXEC_NS, LAST_RESULT
    LAST_EXEC_NS = res.exec_time_ns
    LAST_RESULT = res
    return res.results[0]["out"].reshape(1).astype(np.float32)


# revision 26
# speedup vs baseline: 1.0128x; 1.0128x over previous
"""GCN critic network kernel for 8 TRN2 NeuronCores.

Strategy (dst-shard graph parallel, bf16 gather table, pipelined AllGather):
  - Nodes are permuted host-side into (core, block, slot) bins balancing
    per-block in-degree for each of 4 table quarters (minimizes padding).
  - Each core computes y = dinv * (x @ Wg.T) bf16 for its slice; the slice
    is split into 4 quarter-tables, each AllGathered separately so pass-q
    gather/compute overlaps the AllGather of quarter q+1.
  - Per dst block, segment-sum via one-hot matmuls: one-hot built with
    tensor_scalar is_equal (iota row vs per-edge dst-slot scalar, 4x DVE
    mode); gathered y rows are the moving operand.
  - Self loops are an identity-matmul accumulate of local y (never
    gathered); cross-pass partials park in SBUF (Sacc) via identity-matmul
    reload.
  - v = colsum(relu(.)) via ones-matmul + colsum(x) via DVE row-reduce;
    cross-core reduction as AllGather [8,128] + ones-matmul (cheaper floor
    than AllReduce); tiny MLP with host-pretransposed weights.
"""

import os
import math
import numpy as np
import ml_dtypes

BF16 = ml_dtypes.bfloat16
N = 50000
E = 800000
D = 128
NCORES = 8
NB = 49             # blocks per core
NPC = 6250          # real nodes per core
NPAD = NB * 128     # padded positions per core (6272)
NPASS = 4
QSTART = [0, 12, 24, 36]          # first block of each quarter
QNB = [12, 12, 12, 13]            # blocks per quarter
QROWS = [q * 128 for q in QNB]    # rows per core per quarter
SEGC = int(os.environ.get("KB_SEGC", "8"))    # chunks per gather segment
                                              # (dma_gather hangs above 1024 idxs)
DDS = int(os.environ.get("KB_DDS", "98304"))
NQ = int(os.environ.get("KB_QUEUES", "1"))
PADSLOT = 300.0     # dst-slot sentinel for padding edges (no one-hot match)

DEBUG_BLOCKS = (int(os.environ["KB_DEBUG_BLOCKS"])
                if "KB_DEBUG_BLOCKS" in os.environ else None)
SKIP_MLP = bool(os.environ.get("KB_SKIP_MLP"))
SKIP_MAIN = bool(os.environ.get("KB_SKIP_MAIN"))


def _pack_nodes(inq, classq):
    """Pack nodes into (core, block, slot) bins against per-(pass, block)
    edge-count caps chosen near the ideal chunk budget (cross-core max fill
    sets the uniform chunk count, so bins aim for ct*128 ceilings rather
    than a uniform mean). classq[n] = quarter whose blocks node n must land
    in. Returns c_of, b_of, s_of."""
    c_of = np.empty(N, np.int64)
    b_of = np.empty(N, np.int64)
    s_of = np.empty(N, np.int64)

    for q in range(NPASS):
        nbq = QNB[q]
        nbins = NCORES * nbq
        cap_n = np.full(nbins, 128, np.int64)
        if q == NPASS - 1:
            cap_n[nbq - 1::nbq] = 106          # block 48 of each core
        nodes = np.nonzero(classq == q)[0]
        w = inq[:, nodes].sum(axis=0)
        nodes = nodes[np.argsort(-w, kind="stable")]
        fills = np.zeros((NPASS, nbins))
        cnt = np.zeros(nbins, np.int64)
        tgt = np.maximum(inq.sum(axis=1) / (NCORES * NB), 1.0)  # [NPASS]
        for n in nodes:
            score = ((fills + inq[:, n][:, None]) / tgt[:, None]).max(axis=0)
            score[cnt >= cap_n] = np.inf
            j = int(np.argmin(score))
            c_of[n] = j // nbq
            b_of[n] = QSTART[q] + j % nbq
            s_of[n] = cnt[j]
            fills[:, j] += inq[:, n]
            cnt[j] += 1
    return c_of, b_of, s_of


def _prep(edge_index):
    """Host-side sharding prep. Returns per-core inputs + uniform plan."""
    src = np.asarray(edge_index[0]).astype(np.int64)
    dst = np.asarray(edge_index[1]).astype(np.int64)

    deg_in = np.bincount(dst, minlength=N)
    dinv = (1.0 / np.sqrt(deg_in + 1.0)).astype(np.float32)

    # initial quarter classes: random, sized to real slots per quarter
    sizes = [NCORES * QNB[q] * 128 for q in range(NPASS)]
    sizes[-1] -= NCORES * 22                   # block 48 holds 106 real
    rng = np.random.RandomState(12345)
    perm0 = rng.permutation(N)
    classq = np.empty(N, np.int64)
    p0 = 0
    for q in range(NPASS):
        classq[perm0[p0:p0 + sizes[q]]] = q
        p0 += sizes[q]

    # inq[q, n] = # in-edges of n from quarter-q sources
    inq = np.zeros((NPASS, N), np.int64)
    for q in range(NPASS):
        np.add.at(inq[q], dst[classq[src] == q], 1)

    c_of, b_of, s_of = _pack_nodes(inq, classq)

    # per-quarter table rows (concat over cores), slot-major within a
    # quarter so the y-write DMA gets >=512B contiguous partition lines
    qq = classq                                 # quarter of node (as src)
    bq = b_of - np.asarray(QSTART)[qq]          # block within quarter
    row = c_of * np.asarray(QROWS)[qq] + s_of * np.asarray(QNB)[qq] + bq

    cnt = np.zeros((NCORES, NPASS, NB), np.int64)
    np.add.at(cnt, (c_of[dst], qq[src], b_of[dst]), 1)
    ct = np.ceil(cnt.max(axis=0) / 128.0).astype(np.int64)   # [NPASS, NB]
    ct = np.maximum(ct, 1)
    CH = ct.sum(axis=1)                                      # [NPASS]
    off = np.zeros((NPASS, NB), np.int64)
    off[:, 1:] = np.cumsum(ct, axis=1)[:, :-1]

    def wrap_idx(loc):
        nch = len(loc) // 128
        cols = []
        for s0 in range(0, nch, SEGC):
            seg = loc[s0 * 128: min(nch, s0 + SEGC) * 128]
            a = seg.reshape(-1, 16).T.astype(np.int16)
            cols.append(np.tile(a, (8, 1)))
        return np.concatenate(cols, axis=1)

    in_extra = []
    for c in range(NCORES):
        m = c_of[dst] == c
        es, ed = src[m], dst[m]
        eq = qq[es]
        d = {}
        for P in range(NPASS):
            sel = eq == P
            erow = row[es[sel]]
            eb = b_of[ed[sel]]
            eslot = s_of[ed[sel]]
            order = np.argsort(eb, kind="stable")
            erow, eb, eslot = erow[order], eb[order], eslot[order]
            nslots = int(CH[P]) * 128
            loc = np.zeros(nslots, np.int64)
            dsb = np.full(nslots, PADSLOT, np.float64)
            bstart = np.zeros(NB, np.int64)
            bstart[1:] = np.cumsum(np.bincount(eb, minlength=NB))[:-1]
            pos = off[P][eb] * 128 + (np.arange(len(eb)) - bstart[eb])
            loc[pos] = erow
            dsb[pos] = eslot
            d[f"idx{P}"] = wrap_idx(loc)
            # position i -> (partition i%128, col i//128)
            d[f"dsb{P}"] = np.ascontiguousarray(dsb.reshape(-1, 128).T
                                               ).astype(np.float32)
        in_extra.append(d)

    xt = np.zeros((NCORES, 128, NPAD), dtype=BF16)
    dv = np.zeros((NCORES, 128, NB), np.float32)

    def fill_x(x):
        pos = c_of * NPAD + b_of * 128 + s_of
        xt_flat = np.zeros((NCORES * NPAD, D), np.float32)
        xt_flat[pos] = x
        xtv = xt_flat.reshape(NCORES, NPAD, D)
        for c in range(NCORES):
            xt[c] = np.ascontiguousarray(xtv[c].T).astype(BF16)
        dvf = np.zeros(NCORES * NPAD, np.float32)
        dvf[pos] = dinv
        for c in range(NCORES):
            dv[c] = dvf[c * NPAD:(c + 1) * NPAD].reshape(NB, 128).T

    plan = {"ct": ct, "CH": CH, "off": off}
    return dinv, in_extra, plan, fill_x, xt, dv


def _build(plan, bias_info, probe=False):
    import concourse.bacc as bacc
    import concourse.tile as tile
    from concourse import mybir

    f32 = mybir.dt.float32
    bf16 = mybir.dt.bfloat16
    i16 = mybir.dt.int16
    Alu = mybir.AluOpType
    Act = mybir.ActivationFunctionType
    Ax = mybir.AxisListType

    ct, CH, off = plan["ct"], plan["CH"], plan["off"]
    has_bg, has_b1, has_b2, b3val = bias_info

    nc = bacc.Bacc("TRN2", target_bir_lowering=False, debug=False,
                   num_devices=(1 if probe else NCORES),
                   num_swdge_queues=NQ,
                   dynamic_dma_scratch_size=DDS)

    def din(name, shape, dt=f32):
        return nc.dram_tensor(name, list(shape), dt, kind="ExternalInput")

    xt_d = din("xt", [128, NPAD], bf16)
    dinv_d = din("dinvc", [128, NB])
    iota_d = din("iota2d", [128, 128], bf16)
    ones_d = din("onesh", [128, 1], bf16)
    idf_d = din("idf", [128, 128])
    idh_d = din("idh", [128, 128], bf16)
    wgT_d = din("wgT", [128, 128], bf16)
    w1T_d = din("w1T", [128, 512])
    w2T_d = din("w2T", [128, 4, 256])
    w3T_d = din("w3T", [128, 2])
    idx_d = [din(f"idx{P}", [128, int(CH[P]) * 8], i16) for P in range(NPASS)]
    dsb_d = [din(f"dsb{P}", [128, int(CH[P])]) for P in range(NPASS)]
    bg_d = din("bgt", [128, 128]) if has_bg else None
    b1_d = din("b1c", [128, 4]) if has_b1 else None
    b2_d = din("b2c", [128, 2]) if has_b2 else None
    out_d = nc.dram_tensor("out", [1, 1], f32, kind="ExternalOutput")

    ys = [nc.dram_tensor(f"ys{q}", [QROWS[q], D], bf16) for q in range(NPASS)]
    yf = [nc.dram_tensor(f"yf{q}", [QROWS[q] * NCORES, D], bf16,
                         addr_space="Shared") for q in range(NPASS)]
    vb = nc.dram_tensor("vb", [1, 128], f32)
    vr = nc.dram_tensor("vr", [NCORES, 128], f32, addr_space="Shared")

    RG = [list(range(NCORES))]

    with tile.TileContext(nc) as tc:
        with (
            tc.tile_pool(name="const", bufs=1) as cpool,
            tc.tile_pool(name="xt", bufs=1) as xtpool,
            tc.tile_pool(name="y8", bufs=1) as ypool,
            tc.tile_pool(name="sacc", bufs=1) as apool,
            tc.tile_pool(name="seg", bufs=(2 if SEGC > 8 else 4)) as segpool,
            tc.tile_pool(name="oh", bufs=(2 if SEGC > 8 else 4)) as ohpool,
            tc.tile_pool(name="hb", bufs=3) as hpool,
            tc.tile_pool(name="mlp", bufs=1) as mpool,
            tc.tile_pool(name="psy", bufs=2, space="PSUM") as pbpool,
            tc.tile_pool(name="psS", bufs=4, space="PSUM") as pspool,
            tc.tile_pool(name="psv", bufs=1, space="PSUM") as pvpool,
            tc.tile_pool(name="psT", bufs=1, space="PSUM") as ptpool,
        ):
            # ---- constants ----
            wgT_t = cpool.tile([128, 128], bf16, tag="wgT")
            nc.sync.dma_start(wgT_t[:], wgT_d[:])
            dinv_t = cpool.tile([128, NB], f32, tag="dinv")
            nc.sync.dma_start(dinv_t[:], dinv_d[:])
            iota_t = cpool.tile([128, 128], bf16, tag="iota")
            nc.sync.dma_start(iota_t[:], iota_d[:])
            ones_t = cpool.tile([128, 1], bf16, tag="ones")
            nc.sync.dma_start(ones_t[:], ones_d[:])
            idf_t = cpool.tile([128, 128], f32, tag="idf")
            nc.sync.dma_start(idf_t[:], idf_d[:])
            idh_t = cpool.tile([128, 128], bf16, tag="idh")
            nc.sync.dma_start(idh_t[:], idh_d[:])
            if has_bg:
                bg_t = cpool.tile([128, 128], f32, tag="bgt")
                nc.sync.dma_start(bg_t[:], bg_d[:])

            # ---- phase B: y = dinv * (x @ Wg.T) bf16, 4 quarters ----
            xT = xtpool.tile([128, NPAD], bf16)
            qr0 = [QSTART[q] * 128 for q in range(NPASS)] + [NPAD]
            for q in range(NPASS):
                nc.sync.dma_start(xT[:, qr0[q]:qr0[q + 1]],
                                  xt_d[:, qr0[q]:qr0[q + 1]])
            y8 = ypool.tile([128, NPAD], bf16)

            qend = [QSTART[q] + QNB[q] for q in range(NPASS)]
            for b in range(NB):
                psy = pbpool.tile([128, 128], f32, tag="psy")
                nc.tensor.matmul(psy[:], xT[:, b * 128:(b + 1) * 128],
                                 wgT_t[:], start=True, stop=True)
                nc.scalar.activation(y8[:, b * 128:(b + 1) * 128], psy[:],
                                     Act.Copy, scale=dinv_t[:, b:b + 1])
                q = [qi for qi in range(NPASS) if qend[qi] == b + 1]
                if q:
                    q = q[0]
                    r0 = QSTART[q] * 128
                    nc.sync.dma_start(
                        ys[q][:].rearrange("(p b) d -> p b d", p=128),
                        y8[:, r0:r0 + QROWS[q]]
                        .rearrange("p (b d) -> p b d", d=128))
                    if probe:
                        nc.gpsimd.dma_start(
                            yf[q][0:QROWS[q], :], ys[q][:])
                    else:
                        nc.gpsimd.collective_compute(
                            "AllGather", Alu.bypass, replica_groups=RG,
                            ins=[ys[q][:]], outs=[yf[q][:]])

            # colsum(x) as a column (DVE row-reduce over the transpose)
            vx = mpool.tile([128, 1], f32, tag="vx")
            nc.vector.tensor_reduce(vx[:], xT[:], Ax.X, Alu.add)

            # ---- main gather + one-hot segment-sum, NPASS passes ----
            idx_t = []
            dsb_t = []
            for P in range(NPASS):
                it = cpool.tile([128, int(CH[P]) * 8], i16, tag=f"idx{P}")
                nc.sync.dma_start(it[:], idx_d[P][:])
                idx_t.append(it)
                dt_ = cpool.tile([128, int(CH[P])], f32, tag=f"dsb{P}")
                nc.sync.dma_start(dt_[:], dsb_d[P][:])
                dsb_t.append(dt_)

            Sacc = apool.tile([128, NPAD], f32)
            psv = pvpool.tile([1, 128], f32)

            nblk = (0 if SKIP_MAIN else
                    (NB if DEBUG_BLOCKS is None else DEBUG_BLOCKS))

            def run_pass(P):
                chp = int(CH[P])
                nseg = math.ceil(chp / SEGC)
                seg_tiles = [None] * nseg
                oh_tiles = [None] * nseg
                ptr = [0]

                def ensure(s):
                    while ptr[0] <= s:
                        si = ptr[0]
                        ncols = min(SEGC, chp - si * SEGC)
                        tl = segpool.tile([128, ncols, 128], bf16, tag="seg")
                        nc.gpsimd.dma_gather(
                            tl[:], yf[P][:],
                            idx_t[P][:, si * SEGC * 8:
                                     si * SEGC * 8 + ncols * 8],
                            num_idxs=ncols * 128, num_idxs_reg=ncols * 128,
                            elem_size=128, elem_step=128,
                            queue_num=(si % NQ))
                        oh = ohpool.tile([128, ncols, 128], bf16, tag="oh")
                        for cc in range(ncols):
                            nc.vector.tensor_scalar(
                                oh[:, cc, :], iota_t[:],
                                dsb_t[P][:, si * SEGC + cc:
                                         si * SEGC + cc + 1],
                                None, Alu.is_equal)
                        seg_tiles[si] = tl
                        oh_tiles[si] = oh
                        ptr[0] += 1

                for b in range(nblk):
                    psS = pspool.tile([128, 128], f32, tag="psS")
                    nmm = int(ct[P][b]) + 1
                    if P == 0:
                        # self loop: psS starts with local y rows
                        nc.tensor.matmul(psS[:], idh_t[:],
                                         y8[:, b * 128:(b + 1) * 128],
                                         start=True, stop=(nmm == 1))
                    else:
                        # reload the cross-pass partial
                        nc.tensor.matmul(psS[:], idf_t[:],
                                         Sacc[:, b * 128:(b + 1) * 128],
                                         start=True, stop=(nmm == 1))
                    k = 1
                    for j in range(int(ct[P][b])):
                        ci = int(off[P][b]) + j
                        s, col = divmod(ci, SEGC)
                        ensure(s)
                        nc.tensor.matmul(
                            psS[:], oh_tiles[s][:, col, :],
                            seg_tiles[s][:, col, :],
                            start=False, stop=(k == nmm - 1))
                        k += 1
                    if P < NPASS - 1:
                        nc.scalar.copy(Sacc[:, b * 128:(b + 1) * 128], psS[:])
                    else:
                        if has_bg:
                            tmp = hpool.tile([128, 128], f32, tag="tmp")
                            nc.vector.tensor_tensor(tmp[:], psS[:], bg_t[:],
                                                    Alu.add)
                            src_ap = tmp[:]
                        else:
                            src_ap = psS[:]
                        hb = hpool.tile([128, 128], bf16, tag="hbt")
                        nc.scalar.activation(hb[:], src_ap, Act.Relu,
                                             scale=dinv_t[:, b:b + 1])
                        nc.tensor.matmul(psv[:], ones_t[:], hb[:],
                                         start=(b == 0), stop=(b == nblk - 1),
                                         skip_group_check=True)

            for P in range(NPASS):
                run_pass(P)
            if nblk == 0:
                nc.tensor.matmul(psv[:], ones_t[:],
                                 ones_t[:].to_broadcast([128, 128]),
                                 start=True, stop=True,
                                 skip_group_check=True)

            # ---- v = colsum(h) + colsum(x); AllReduce ----
            vh = mpool.tile([1, 128], f32, tag="vh")
            nc.scalar.copy(vh[:], psv[:])
            pvx = ptpool.tile([1, 128], f32, tag="pst")
            nc.tensor.transpose(pvx[:], vx[:], idf_t[:])
            vrow = mpool.tile([1, 128], f32, tag="vrow")
            nc.vector.tensor_tensor(vrow[:], vh[:], pvx[:], Alu.add)
            nc.sync.dma_start(vb[:], vrow[:])
            if probe:
                nc.gpsimd.dma_start(vr[0:1, :], vb[:])
            else:
                nc.gpsimd.collective_compute(
                    "AllGather", Alu.bypass, replica_groups=RG,
                    ins=[vb[:]], outs=[vr[:]])
            vfull8 = mpool.tile([NCORES, 128], f32, tag="vfull8")
            nc.sync.dma_start(vfull8[:], vr[:])
            ones8 = mpool.tile([NCORES, 1], f32, tag="ones8")
            nc.vector.memset(ones8[:], 1.0)
            psum_v = ptpool.tile([1, 128], f32, tag="pst")
            nc.tensor.matmul(psum_v[:], ones8[:], vfull8[:],
                             start=True, stop=True)
            vfull = mpool.tile([1, 128], f32, tag="vfull")
            nc.scalar.copy(vfull[:], psum_v[:])

            # ---- MLP head (host-pretransposed weights) ----
            if SKIP_MLP:
                nc.sync.dma_start(out_d[:], vfull[0:1, 0:1])
            else:
                w1T_t = cpool.tile([128, 512], f32, tag="w1T")
                nc.sync.dma_start(w1T_t[:], w1T_d[:])
                w2T_t = cpool.tile([128, 4, 256], f32, tag="w2T")
                nc.sync.dma_start(w2T_t[:], w2T_d[:])
                w3T_t = cpool.tile([128, 2], f32, tag="w3T")
                nc.sync.dma_start(w3T_t[:], w3T_d[:])
                if has_b1:
                    b1_t = cpool.tile([128, 4], f32, tag="b1c")
                    nc.sync.dma_start(b1_t[:], b1_d[:])
                if has_b2:
                    b2_t = cpool.tile([128, 2], f32, tag="b2c")
                    nc.sync.dma_start(b2_t[:], b2_d[:])

                pvc = ptpool.tile([128, 1], f32, tag="pst")
                nc.tensor.transpose(pvc[:], vfull[:], idf_t[0:1, 0:1])
                vcol = mpool.tile([128, 1], f32, tag="vcol")
                nc.vector.tensor_copy(vcol[:], pvc[:])

                a1 = []
                for m in range(4):
                    ps1 = ptpool.tile([128, 1], f32, tag="pst")
                    nc.tensor.matmul(ps1[:], w1T_t[:, m * 128:(m + 1) * 128],
                                     vcol[:], start=True, stop=True)
                    a1t = mpool.tile([128, 1], f32, tag=f"a1{m}")
                    if has_b1:
                        nc.scalar.activation(a1t[:], ps1[:], Act.Relu,
                                             bias=b1_t[:, m:m + 1])
                    else:
                        nc.scalar.activation(a1t[:], ps1[:], Act.Relu)
                    a1.append(a1t)

                a2 = []
                for m in range(2):
                    ps2 = ptpool.tile([128, 1], f32, tag="pst")
                    for kk in range(4):
                        nc.tensor.matmul(
                            ps2[:], w2T_t[:, kk, m * 128:(m + 1) * 128],
                            a1[kk][:], start=(kk == 0), stop=(kk == 3))
                    a2t = mpool.tile([128, 1], f32, tag=f"a2{m}")
                    if has_b2:
                        nc.scalar.activation(a2t[:], ps2[:], Act.Relu,
                                             bias=b2_t[:, m:m + 1])
                    else:
                        nc.scalar.activation(a2t[:], ps2[:], Act.Relu)
                    a2.append(a2t)

                ps3 = ptpool.tile([1, 1], f32, tag="pst")
                for kk in range(2):
                    nc.tensor.matmul(ps3[:], w3T_t[:, kk:kk + 1], a2[kk][:],
                                     start=(kk == 0), stop=(kk == 1))
                ot = mpool.tile([1, 1], f32, tag="ot")
                nc.scalar.activation(ot[:], ps3[:], Act.Copy,
                                     bias=float(b3val))
                nc.sync.dma_start(out_d[:], ot[:])

    nc.compile()
    return nc


TRACE = False
LAST_EXEC_NS = None
LAST_RESULT = None


def kernel(**inputs):
    from concourse.bass_utils import run_bass_kernel_spmd

    x = np.asarray(inputs["x"], dtype=np.float32)
    Wg = np.asarray(inputs["Wg"], dtype=np.float32)
    bg = np.asarray(inputs["bg"], dtype=np.float32)
    W1 = np.asarray(inputs["W1"], dtype=np.float32)
    b1 = np.asarray(inputs["b1"], dtype=np.float32)
    W2 = np.asarray(inputs["W2"], dtype=np.float32)
    b2 = np.asarray(inputs["b2"], dtype=np.float32)
    W3 = np.asarray(inputs["W3"], dtype=np.float32)
    b3 = np.asarray(inputs["b3"], dtype=np.float32)

    dinv, in_extra, plan, fill_x, xt, dv = _prep(inputs["edge_index"])
    fill_x(x)
    bias_info = (bool(bg.any()), bool(b1.any()), bool(b2.any()),
                 float(b3.reshape(-1)[0]))
    nc = _build(plan, bias_info)

    iota = np.tile(np.arange(128, dtype=np.float32)[None, :],
                   (128, 1)).astype(BF16)
    idf = np.eye(128, dtype=np.float32)
    idh = np.eye(128, dtype=np.float32).astype(BF16)
    ones = np.ones((128, 1), dtype=np.float32).astype(BF16)
    w1T = np.ascontiguousarray(W1.T)
    w2T = np.ascontiguousarray(W2.T).reshape(4, 128, 256).transpose(1, 0, 2)
    w2T = np.ascontiguousarray(w2T)
    w3T = np.ascontiguousarray(W3.reshape(256)).reshape(2, 128).T
    w3T = np.ascontiguousarray(w3T)

    in_maps = []
    for c in range(NCORES):
        m = {"xt": xt[c], "dinvc": dv[c], "iota2d": iota, "onesh": ones,
             "idf": idf, "idh": idh,
             "wgT": np.ascontiguousarray(Wg.T).astype(BF16),
             "w1T": w1T, "w2T": w2T, "w3T": w3T}
        for P in range(NPASS):
            m[f"idx{P}"] = in_extra[c][f"idx{P}"]
            m[f"dsb{P}"] = in_extra[c][f"dsb{P}"]
        if bias_info[0]:
            m["bgt"] = np.tile(bg.reshape(1, 128), (128, 1))
        if bias_info[1]:
            m["b1c"] = np.ascontiguousarray(b1.reshape(4, 128).T)
        if bias_info[2]:
            m["b2c"] = np.ascontiguousarray(b2.reshape(2, 128).T)
        in_maps.append(m)

    res = run_bass_kernel_spmd(nc, in_maps, list(range(NCORES)), trace=TRACE)
    global LAST_EXEC_NS, LAST_RESULT
    LAST_EXEC_NS = res.exec_time_ns
    LAST_RESULT = res
    return res.results[0]["out"].reshape(1).astype(np.float32)
